# revision 3
# baseline (speedup 1.0000x reference)
"""Bahdanau attention Trainium2 kernel.

reference:
    h_vals = values @ W1 + b1                      # [B,T,U]
    h_query = query @ W2 + b2                      # [B,U]
    score = tanh(h_vals + h_query[:,None,:]) @ V + bV   # [B,T,1]
    attn = softmax(score, axis=1)                  # [B,T,1]
    ctx = sum(attn * values, axis=1)               # [B,D]
    returns (ctx, attn)

B=32, T=4096, D=512, U=256.  Data-parallel: batch sharded 4-per-core over
8 NeuronCores; small weights replicated.

Host-side prep (per core):
  - hq = query @ W2 + b2 + b1  (tiny, fp32) packed as per-partition bias
  - values passed twice in bf16: natural [T,D] (context matmul) and
    transposed [D,T] (values@W1 matmul). Same total HBM bytes as fp32-once,
    and no on-chip transpose is needed.
  - softmax is shift-invariant => bV dropped; no max-subtraction needed
    (|score| <= sum|V| ~= 13, exp stays in fp32 range).

Device per batch:
  mm1:   hT[u,t] = W1.T @ valuesT   (W1-chunk stationary, bf16, PSUM f32)
  tanh:  ACT, bias = hq per-partition, out bf16 [u,t]
  score: PE, lhsT = tanh tile [u128, t128], rhs = V chunk [u128,1]
         -> score psum [128, 32] (partition = t%128, col = t//128)
  softmax: ACT exp with fused row-sum, PE ones-matmul for total,
         DVE reciprocal, PE rank-1 broadcast, DVE scale (+bf16 copy)
  ctx:   PE, lhsT = attn col [t128,1], rhs = values natural [t128, d512]
"""

import sys

sys.path.insert(0, "/opt/trn_rl_repo")

import numpy as np
import ml_dtypes

import concourse.bacc as bacc
import concourse.mybir as mybir
import concourse.tile as tile
from concourse.bass_utils import run_bass_kernel_spmd

B, T, D, U = 32, 4096, 512, 256
NCORES = 8
NB = B // NCORES          # batches per core
TW = 512                  # mm1 t-window (one PSUM bank)
NTW = T // TW             # 8
NCH = T // 128            # 32 t-chunks of 128
ND = D // 128             # 4 d-chunks
NU = U // 128             # 2 u-chunks

BF16 = mybir.dt.bfloat16
F32 = mybir.dt.float32
AF = mybir.ActivationFunctionType

_cache = {}


def build_nc():
    nc = bacc.Bacc("TRN2", target_bir_lowering=False, debug=False)

    vT = nc.declare_dram_parameter("vT", [NB, D, T], BF16, isOutput=False)
    vN = nc.declare_dram_parameter("vN", [NB, T, D], BF16, isOutput=False)
    w1 = nc.declare_dram_parameter("w1", [D, U], BF16, isOutput=False)
    hq = nc.declare_dram_parameter("hq", [128, NB * NU], F32, isOutput=False)
    vv = nc.declare_dram_parameter("vv", [128, NU], BF16, isOutput=False)
    ctx_out = nc.declare_dram_parameter("ctx_out", [NB, D], F32, isOutput=True)
    attn_out = nc.declare_dram_parameter(
        "attn_out", [NB, 128, NCH], F32, isOutput=True
    )

    with tile.TileContext(nc) as tc:
        with (
            tc.tile_pool(name="const", bufs=1) as constp,
            tc.tile_pool(name="vt", bufs=2) as vtp,
            tc.tile_pool(name="vn", bufs=2) as vnp,
            tc.tile_pool(name="th", bufs=4) as thp,
            tc.tile_pool(name="small", bufs=2) as smallp,
            tc.tile_pool(name="mm", bufs=3, space="PSUM") as mmp,
            tc.tile_pool(name="scoreps", bufs=2, space="PSUM") as scorep,
            tc.tile_pool(name="miscps", bufs=1, space="PSUM") as miscp,
        ):
            w1_sb = constp.tile([128, ND * U], BF16)
            nc.sync.dma_start(
                w1_sb[:].rearrange("p (n u) -> p n u", u=U),
                w1.ap().rearrange("(n p) u -> p n u", p=128),
            )
            hq_sb = constp.tile([128, NB * NU], F32)
            nc.sync.dma_start(hq_sb[:], hq[:, :])
            vv_sb = constp.tile([128, NU], BF16)
            nc.sync.dma_start(vv_sb[:], vv[:, :])
            ones_col = constp.tile([128, 1], F32)
            nc.vector.memset(ones_col[:], 1.0)
            ones_row = constp.tile([1, 128], F32)
            nc.vector.memset(ones_row[:], 1.0)

            for b in range(NB):
                # big streaming loads: [128, c*T + t] and [128, n*D + d]
                vt_sb = vtp.tile([128, ND * T], BF16)
                nc.sync.dma_start(
                    vt_sb[:].rearrange("p (c t) -> p c t", t=T),
                    vT[b].rearrange("(c p) t -> p c t", p=128),
                )
                vn_sb = vnp.tile([128, NCH * D], BF16)
                nc.sync.dma_start(
                    vn_sb[:].rearrange("p (n d) -> p n d", d=D),
                    vN[b].rearrange("(n p) d -> p n d", p=128),
                )

                score_ps = scorep.tile([128, NCH], F32)
                for w in range(NTW):
                    ths = []
                    for u in range(NU):
                        ps = mmp.tile([128, TW], F32)
                        for n in range(ND):
                            nc.tensor.matmul(
                                ps[:],
                                w1_sb[:, n * U + u * 128 : n * U + u * 128 + 128],
                                vt_sb[:, n * T + w * TW : n * T + w * TW + TW],
                                start=(n == 0),
                                stop=(n == ND - 1),
                            )
                        th = thp.tile([128, TW], BF16)
                        nc.scalar.activation(
                            th[:],
                            ps[:],
                            AF.Tanh,
                            bias=hq_sb[:, b * NU + u : b * NU + u + 1],
                        )
                        ths.append(th)
                    for q in range(TW // 128):
                        j = w * (TW // 128) + q
                        for u in range(NU):
                            nc.tensor.matmul(
                                score_ps[:, j : j + 1],
                                ths[u][:, q * 128 : (q + 1) * 128],
                                vv_sb[:, u : u + 1],
                                start=(u == 0),
                                stop=(u == NU - 1),
                                skip_group_check=True,
                            )

                # ---- softmax over all T (partition x col layout) ----
                exp_sb = smallp.tile([128, NCH], F32)
                rowsum = smallp.tile([128, 1], F32)
                nc.scalar.activation(
                    exp_sb[:], score_ps[:], AF.Exp, accum_out=rowsum[:]
                )
                tot_ps = miscp.tile([1, 1], F32)
                nc.tensor.matmul(
                    tot_ps[:], ones_col[:], rowsum[:], start=True, stop=True
                )
                inv_sb = smallp.tile([1, 1], F32)
                nc.vector.reciprocal(inv_sb[:], tot_ps[:])
                bcast_ps = miscp.tile([128, 1], F32)
                nc.tensor.matmul(
                    bcast_ps[:], ones_row[:], inv_sb[:], start=True, stop=True
                )
                inv_bc = smallp.tile([128, 1], F32)
                nc.vector.tensor_copy(inv_bc[:], bcast_ps[:])
                attn_f = smallp.tile([128, NCH], F32)
                nc.vector.tensor_scalar_mul(attn_f[:], exp_sb[:], inv_bc[:])
                attn_b = smallp.tile([128, NCH], BF16)
                nc.vector.tensor_scalar_mul(attn_b[:], exp_sb[:], inv_bc[:])
                nc.scalar.dma_start(attn_out[b], attn_f[:])

                # ---- context ----
                ctx_ps = miscp.tile([1, D], F32)
                for j in range(NCH):
                    nc.tensor.matmul(
                        ctx_ps[:],
                        attn_b[:, j : j + 1],
                        vn_sb[:, j * D : (j + 1) * D],
                        start=(j == 0),
                        stop=(j == NCH - 1),
                    )
                ctx_sb = smallp.tile([1, D], F32)
                nc.vector.tensor_copy(ctx_sb[:], ctx_ps[:])
                nc.scalar.dma_start(ctx_out[b], ctx_sb[:])

    nc.compile()
    return nc


def get_nc():
    if "nc" not in _cache:
        _cache["nc"] = build_nc()
    return _cache["nc"]


def make_in_maps(values, query, W1, b1, W2, b2, V, bV):
    values = np.asarray(values, dtype=np.float32)
    query = np.asarray(query, dtype=np.float32)
    W1 = np.asarray(W1, dtype=np.float32)
    b1 = np.asarray(b1, dtype=np.float32)
    W2 = np.asarray(W2, dtype=np.float32)
    b2 = np.asarray(b2, dtype=np.float32)
    V = np.asarray(V, dtype=np.float32)

    hq_all = query @ W2 + b2 + b1  # [B, U] fp32
    w1_bf = W1.astype(ml_dtypes.bfloat16)
    vv_dev = np.ascontiguousarray(V.reshape(NU, 128).T.astype(ml_dtypes.bfloat16))

    in_maps = []
    for c in range(NCORES):
        sl = slice(c * NB, (c + 1) * NB)
        vals_c = values[sl]
        vT_c = np.ascontiguousarray(vals_c.transpose(0, 2, 1)).astype(
            ml_dtypes.bfloat16
        )
        vN_c = vals_c.astype(ml_dtypes.bfloat16)
        hq_c = hq_all[sl]  # [NB, U]
        # hq_dev[p, b*NU+u] = hq_c[b, u*128+p]
        hq_dev = np.ascontiguousarray(
            hq_c.reshape(NB * NU, 128).T.astype(np.float32)
        )
        in_maps.append(
            {
                "vT": vT_c,
                "vN": vN_c,
                "w1": w1_bf,
                "hq": hq_dev,
                "vv": vv_dev,
            }
        )
    return in_maps


def postprocess(results):
    ctx = np.concatenate([r["ctx_out"] for r in results], axis=0).astype(np.float32)
    attn = np.concatenate(
        [
            r["attn_out"].transpose(0, 2, 1).reshape(NB, T, 1)
            for r in results
        ],
        axis=0,
    ).astype(np.float32)
    return ctx, attn


def kernel(values, query, W1, b1, W2, b2, V, bV):
    nc = get_nc()
    in_maps = make_in_maps(values, query, W1, b1, W2, b2, V, bV)
    res = run_bass_kernel_spmd(nc, in_maps, list(range(NCORES)))
    return postprocess(res.results)


# revision 15
# speedup vs baseline: 1.0832x; 1.0832x over previous
"""Bahdanau attention Trainium2 kernel.

reference:
    h_vals = values @ W1 + b1                      # [B,T,U]
    h_query = query @ W2 + b2                      # [B,U]
    score = tanh(h_vals + h_query[:,None,:]) @ V + bV   # [B,T,1]
    attn = softmax(score, axis=1)                  # [B,T,1]
    ctx = sum(attn * values, axis=1)               # [B,D]
    returns (ctx, attn)

B=32, T=4096, D=512, U=256.  Data-parallel: batch sharded 4-per-core over
8 NeuronCores; small weights replicated.

Host-side prep (per core):
  - hq = query @ W2 + b2 + b1  (tiny, fp32) packed as per-partition bias
  - values passed twice in bf16: natural [T,D] (context matmul) and
    transposed [D,T] (values@W1 matmul). Same total HBM bytes as fp32-once,
    and no on-chip transpose is needed.
  - softmax is shift-invariant => bV dropped; no max-subtraction needed
    (|score| <= sum|V| ~= 13, exp stays in fp32 range).

Device per batch:
  mm1:   hT[u,t] = W1.T @ valuesT   (W1-chunk stationary, bf16, PSUM f32)
  tanh:  ACT, bias = hq per-partition, out bf16 [u,t]
  score: PE, lhsT = tanh tile [u128, t128], rhs = V chunk [u128,1]
         -> score psum [128, 32] (partition = t%128, col = t//128)
  softmax: ACT exp with fused row-sum, PE ones-matmul for total,
         DVE reciprocal, PE rank-1 broadcast, DVE scale (+bf16 copy)
  ctx:   PE, lhsT = attn col [t128,1], rhs = values natural [t128, d512]
"""

import sys

sys.path.insert(0, "/opt/trn_rl_repo")

import numpy as np
import ml_dtypes

import concourse.bacc as bacc
import concourse.mybir as mybir
import concourse.tile as tile
from concourse.bass_utils import run_bass_kernel_spmd

B, T, D, U = 32, 4096, 512, 256
NCORES = 8
NB = B // NCORES          # batches per core
TW = 512                  # mm1 t-window (one PSUM bank)
NTW = T // TW             # 8
NCH = T // 128            # 32 t-chunks of 128
ND = D // 128             # 4 d-chunks
NU = U // 128             # 2 u-chunks

BF16 = mybir.dt.bfloat16
F32 = mybir.dt.float32
AF = mybir.ActivationFunctionType

_cache = {}


def build_nc():
    nc = bacc.Bacc("TRN2", target_bir_lowering=False, debug=False)

    vT = nc.declare_dram_parameter("vT", [NB, D, T], BF16, isOutput=False)
    vN = nc.declare_dram_parameter("vN", [NB, T, D], BF16, isOutput=False)
    w1 = nc.declare_dram_parameter("w1", [D, U], BF16, isOutput=False)
    hq = nc.declare_dram_parameter("hq", [128, NB * NU], F32, isOutput=False)
    vv = nc.declare_dram_parameter("vv", [128, NU], BF16, isOutput=False)
    ctx_out = nc.declare_dram_parameter("ctx_out", [NB, D], F32, isOutput=True)
    attn_out = nc.declare_dram_parameter(
        "attn_out", [NB, 128, NCH], F32, isOutput=True
    )

    with tile.TileContext(nc) as tc:
        with (
            tc.tile_pool(name="const", bufs=1) as constp,
            tc.tile_pool(name="vt", bufs=8) as vtp,
            tc.tile_pool(name="vn", bufs=2) as vnp,
            tc.tile_pool(name="th", bufs=4) as thp,
            tc.tile_pool(name="small", bufs=2) as smallp,
            tc.tile_pool(name="mm", bufs=4, space="PSUM") as mmp,
            tc.tile_pool(name="scoreps", bufs=2, space="PSUM") as scorep,
            tc.tile_pool(name="miscps", bufs=1, space="PSUM") as miscp,
        ):
            w1_sb = constp.tile([128, ND * U], BF16)
            nc.sync.dma_start(
                w1_sb[:].rearrange("p (n u) -> p n u", u=U),
                w1.ap().rearrange("(n p) u -> p n u", p=128),
            )
            hq_sb = constp.tile([128, NB * NU], F32)
            nc.sync.dma_start(hq_sb[:], hq[:, :])
            vv_sb = constp.tile([128, NU], BF16)
            nc.sync.dma_start(vv_sb[:], vv[:, :])
            ones_col = constp.tile([128, 1], F32)
            nc.vector.memset(ones_col[:], 1.0)
            ones_row = constp.tile([1, 128], F32)
            nc.vector.memset(ones_row[:], 1.0)

            for b in range(NB):
                # big streaming loads: [128, c*T + t] and [128, n*D + d]
                # vT loaded as separate t-slice tiles so mm1 w=0 can start
                # after ~1MB instead of the full 4MB
                NSLICE = 4
                SLW = T // NSLICE
                vt_slices = []
                for s in range(NSLICE):
                    vt_s = vtp.tile([128, ND * SLW], BF16, tag="vt_s")
                    nc.sync.dma_start(
                        vt_s[:].rearrange("p (c t) -> p c t", t=SLW),
                        vT[b].rearrange("(c p) t -> p c t", p=128)[
                            :, :, s * SLW : (s + 1) * SLW
                        ],
                    )
                    vt_slices.append(vt_s)
                # vN on the other HWDGE ring (ACT) so it doesn't queue ahead
                # of the next batch's vT slices
                # natural-layout values with t = w*512 + p*4 + q mapping:
                # partition p holds contiguous 4-row runs -> 4KB descriptors.
                # chunk j = w*4+q pairs column j of the score/attn tiles with
                # vn free-slice j; both use the same t-set by construction.
                vn_sb = vnp.tile([128, NCH * D], BF16)
                nc.scalar.dma_start(
                    vn_sb[:].rearrange("p (w q d) -> p w q d", q=4, d=D),
                    vN[b].rearrange("(w p q) d -> p w q d", p=128, q=4),
                )

                score_ps = scorep.tile([128, NCH], F32)
                for w in range(NTW):
                    ths = []
                    for u in range(NU):
                        ps = mmp.tile([128, TW], F32)
                        vt_s = vt_slices[w * TW // SLW]
                        wo = (w * TW) % SLW
                        for n in range(ND):
                            nc.tensor.matmul(
                                ps[:],
                                w1_sb[:, n * U + u * 128 : n * U + u * 128 + 128],
                                vt_s[:, n * SLW + wo : n * SLW + wo + TW],
                                start=(n == 0),
                                stop=(n == ND - 1),
                            )
                        th = thp.tile([128, TW], BF16)
                        nc.scalar.activation(
                            th[:],
                            ps[:],
                            AF.Tanh,
                            bias=hq_sb[:, b * NU + u : b * NU + u + 1],
                        )
                        ths.append(th)
                    for q in range(TW // 128):
                        j = w * (TW // 128) + q
                        for u in range(NU):
                            # within-window t for column j=(w,q) is p*4+q:
                            # stride-4 view of the tanh tile
                            lhsT = ths[u].rearrange("u (p q) -> u q p", q=4)[
                                :, q, :
                            ]
                            nc.tensor.matmul(
                                score_ps[:, j : j + 1],
                                lhsT,
                                vv_sb[:, u : u + 1],
                                start=(u == 0),
                                stop=(u == NU - 1),
                                skip_group_check=True,
                            )

                # ---- softmax over all T (partition x col layout) ----
                exp_sb = smallp.tile([128, NCH], F32)
                rowsum = smallp.tile([128, 1], F32)
                nc.scalar.activation(
                    exp_sb[:], score_ps[:], AF.Exp, accum_out=rowsum[:]
                )
                tot_ps = miscp.tile([1, 1], F32, tag="small_ps", padded_shape=[128, 1])
                nc.tensor.matmul(
                    tot_ps[:], ones_col[:], rowsum[:], start=True, stop=True
                )
                inv_sb = smallp.tile([1, 1], F32)
                nc.vector.reciprocal(inv_sb[:], tot_ps[:])
                bcast_ps = miscp.tile([128, 1], F32, tag="small_ps")
                nc.tensor.matmul(
                    bcast_ps[:], ones_row[:], inv_sb[:], start=True, stop=True
                )
                inv_bc = smallp.tile([128, 1], F32)
                nc.vector.tensor_copy(inv_bc[:], bcast_ps[:])
                attn_f = smallp.tile([128, NCH], F32)
                nc.vector.tensor_scalar_mul(attn_f[:], exp_sb[:], inv_bc[:])
                attn_b = smallp.tile([128, NCH], BF16)
                nc.vector.tensor_scalar_mul(attn_b[:], exp_sb[:], inv_bc[:])
                nc.scalar.dma_start(attn_out[b], attn_f[:])

                # ---- context ----
                # 4 concurrent M=1 matmuls in separate 32-col PE groups
                # (tile_position col-tiling); partial sums land on psum
                # partitions 0/32/64/96 and are summed on DVE.
                ctx_ps = miscp.tile([128, D], F32, tag="ctx_ps")
                NG = 4
                for j in range(NCH):
                    g = j % NG
                    nc.tensor.matmul(
                        ctx_ps[32 * g : 32 * g + 1, :],
                        attn_b[:, j : j + 1],
                        vn_sb[:, j * D : (j + 1) * D],
                        start=(j < NG),
                        stop=(j >= NCH - NG),
                        tile_position=(0, 32 * g),
                        skip_group_check=True,
                    )
                # only one PSUM operand allowed per DVE op: copy then chain adds
                ctx_sb = smallp.tile([1, D], F32)
                nc.vector.tensor_copy(ctx_sb[:], ctx_ps[0:1, :])
                for g in range(1, NG):
                    nc.vector.tensor_tensor(
                        ctx_sb[:], ctx_sb[:], ctx_ps[32 * g : 32 * g + 1, :],
                        mybir.AluOpType.add,
                    )
                nc.scalar.dma_start(ctx_out[b], ctx_sb[:])

    nc.compile()
    return nc


def get_nc():
    if "nc" not in _cache:
        _cache["nc"] = build_nc()
    return _cache["nc"]


def make_in_maps(values, query, W1, b1, W2, b2, V, bV):
    values = np.asarray(values, dtype=np.float32)
    query = np.asarray(query, dtype=np.float32)
    W1 = np.asarray(W1, dtype=np.float32)
    b1 = np.asarray(b1, dtype=np.float32)
    W2 = np.asarray(W2, dtype=np.float32)
    b2 = np.asarray(b2, dtype=np.float32)
    V = np.asarray(V, dtype=np.float32)

    hq_all = query @ W2 + b2 + b1  # [B, U] fp32
    w1_bf = W1.astype(ml_dtypes.bfloat16)
    vv_dev = np.ascontiguousarray(V.reshape(NU, 128).T.astype(ml_dtypes.bfloat16))

    in_maps = []
    for c in range(NCORES):
        sl = slice(c * NB, (c + 1) * NB)
        vals_c = values[sl]
        vT_c = np.ascontiguousarray(vals_c.transpose(0, 2, 1)).astype(
            ml_dtypes.bfloat16
        )
        vN_c = vals_c.astype(ml_dtypes.bfloat16)
        hq_c = hq_all[sl]  # [NB, U]
        # hq_dev[p, b*NU+u] = hq_c[b, u*128+p]
        hq_dev = np.ascontiguousarray(
            hq_c.reshape(NB * NU, 128).T.astype(np.float32)
        )
        in_maps.append(
            {
                "vT": vT_c,
                "vN": vN_c,
                "w1": w1_bf,
                "hq": hq_dev,
                "vv": vv_dev,
            }
        )
    return in_maps


def postprocess(results):
    ctx = np.concatenate([r["ctx_out"] for r in results], axis=0).astype(np.float32)
    # attn_out[b, p, w*4+q] holds weight for t = w*512 + p*4 + q
    attn = np.concatenate(
        [
            r["attn_out"]
            .reshape(NB, 128, NTW, 4)
            .transpose(0, 2, 1, 3)
            .reshape(NB, T, 1)
            for r in results
        ],
        axis=0,
    ).astype(np.float32)
    return ctx, attn


def kernel(values, query, W1, b1, W2, b2, V, bV):
    nc = get_nc()
    in_maps = make_in_maps(values, query, W1, b1, W2, b2, V, bV)
    res = run_bass_kernel_spmd(nc, in_maps, list(range(NCORES)))
    return postprocess(res.results)


# revision 18
# speedup vs baseline: 1.2331x; 1.1384x over previous
"""Bahdanau attention Trainium2 kernel.

reference:
    h_vals = values @ W1 + b1                      # [B,T,U]
    h_query = query @ W2 + b2                      # [B,U]
    score = tanh(h_vals + h_query[:,None,:]) @ V + bV   # [B,T,1]
    attn = softmax(score, axis=1)                  # [B,T,1]
    ctx = sum(attn * values, axis=1)               # [B,D]
    returns (ctx, attn)

B=32, T=4096, D=512, U=256.  Data-parallel: batch sharded 4-per-core over
8 NeuronCores; small weights replicated.

Host-side prep (per core):
  - hq = query @ W2 + b2 + b1  (tiny, fp32) packed as per-partition bias
  - values passed twice in bf16: natural [T,D] (context matmul) and
    transposed [D,T] (values@W1 matmul). Same total HBM bytes as fp32-once,
    and no on-chip transpose is needed.
  - softmax is shift-invariant => bV dropped; no max-subtraction needed
    (|score| <= sum|V| ~= 13, exp stays in fp32 range).

Device per batch:
  mm1:   hT[u,t] = W1.T @ valuesT   (W1-chunk stationary, bf16, PSUM f32)
  tanh:  ACT, bias = hq per-partition, out bf16 [u,t]
  score: PE, lhsT = tanh tile [u128, t128], rhs = V chunk [u128,1]
         -> score psum [128, 32] (partition = t%128, col = t//128)
  softmax: ACT exp with fused row-sum, PE ones-matmul for total,
         DVE reciprocal, PE rank-1 broadcast, DVE scale (+bf16 copy)
  ctx:   PE, lhsT = attn col [t128,1], rhs = values natural [t128, d512]
"""

import sys

sys.path.insert(0, "/opt/trn_rl_repo")

import numpy as np
import ml_dtypes

import concourse.bacc as bacc
import concourse.mybir as mybir
import concourse.tile as tile
from concourse.bass_utils import run_bass_kernel_spmd

B, T, D, U = 32, 4096, 512, 256
NCORES = 8
NB = B // NCORES          # batches per core
TW = 512                  # mm1 t-window (one PSUM bank)
NTW = T // TW             # 8
NCH = T // 128            # 32 t-chunks of 128
ND = D // 128             # 4 d-chunks
NU = U // 128             # 2 u-chunks

BF16 = mybir.dt.bfloat16
F32 = mybir.dt.float32
AF = mybir.ActivationFunctionType

_cache = {}


def build_nc():
    nc = bacc.Bacc("TRN2", target_bir_lowering=False, debug=False)

    vT = nc.declare_dram_parameter("vT", [NB, D, T], BF16, isOutput=False)
    vN = nc.declare_dram_parameter("vN", [NB, T, D], BF16, isOutput=False)
    w1 = nc.declare_dram_parameter("w1", [D, U], BF16, isOutput=False)
    hq = nc.declare_dram_parameter("hq", [128, NB * NU], F32, isOutput=False)
    vv = nc.declare_dram_parameter("vv", [128, NU], BF16, isOutput=False)
    ctx_out = nc.declare_dram_parameter("ctx_out", [NB, D], F32, isOutput=True)
    attn_out = nc.declare_dram_parameter(
        "attn_out", [NB, 128, NCH], F32, isOutput=True
    )

    with tile.TileContext(nc) as tc:
        with (
            tc.tile_pool(name="const", bufs=1) as constp,
            tc.tile_pool(name="vt", bufs=4) as vtp,
            tc.tile_pool(name="vn", bufs=2) as vnp,
            tc.tile_pool(name="th", bufs=4) as thp,
            tc.tile_pool(name="small", bufs=2) as smallp,
            tc.tile_pool(name="mm", bufs=4, space="PSUM") as mmp,
            tc.tile_pool(name="scoreps", bufs=2, space="PSUM") as scorep,
            tc.tile_pool(name="miscps", bufs=1, space="PSUM") as miscp,
        ):
            w1_sb = constp.tile([128, ND * U], BF16)
            nc.sync.dma_start(
                w1_sb[:].rearrange("p (n u) -> p n u", u=U),
                w1.ap().rearrange("(n p) u -> p n u", p=128),
            )
            hq_sb = constp.tile([128, NB * NU], F32)
            nc.sync.dma_start(hq_sb[:], hq[:, :])
            vv_sb = constp.tile([128, NU], BF16)
            nc.sync.dma_start(vv_sb[:], vv[:, :])
            ones_col = constp.tile([128, 1], F32)
            nc.vector.memset(ones_col[:], 1.0)
            ones_row = constp.tile([1, 128], F32)
            nc.vector.memset(ones_row[:], 1.0)

            for b in range(NB):
                # big streaming loads: [128, c*T + t] and [128, n*D + d]
                # vT loaded as separate t-slice tiles so mm1 w=0 can start
                # after ~1MB instead of the full 4MB
                NSLICE = 2
                SLW = T // NSLICE
                vt_slices = []
                for s in range(NSLICE):
                    vt_s = vtp.tile([128, ND * SLW], BF16, tag="vt_s")
                    nc.sync.dma_start(
                        vt_s[:].rearrange("p (c t) -> p c t", t=SLW),
                        vT[b].rearrange("(c p) t -> p c t", p=128)[
                            :, :, s * SLW : (s + 1) * SLW
                        ],
                    )
                    vt_slices.append(vt_s)
                # vN on the other HWDGE ring (ACT) so it doesn't queue ahead
                # of the next batch's vT slices
                # natural-layout values with t = w*512 + p*4 + q mapping:
                # partition p holds contiguous 4-row runs -> 4KB descriptors.
                # chunk j = w*4+q pairs column j of the score/attn tiles with
                # vn free-slice j; both use the same t-set by construction.
                vn_sb = vnp.tile([128, NCH * D], BF16)
                nc.sync.dma_start(
                    vn_sb[:].rearrange("p (w q d) -> p w q d", q=4, d=D),
                    vN[b].rearrange("(w p q) d -> p w q d", p=128, q=4),
                )

                score_ps = scorep.tile([128, NCH], F32)
                for w in range(NTW):
                    ths = []
                    for u in range(NU):
                        ps = mmp.tile([128, TW], F32)
                        vt_s = vt_slices[w * TW // SLW]
                        wo = (w * TW) % SLW
                        for n in range(ND):
                            nc.tensor.matmul(
                                ps[:],
                                w1_sb[:, n * U + u * 128 : n * U + u * 128 + 128],
                                vt_s[:, n * SLW + wo : n * SLW + wo + TW],
                                start=(n == 0),
                                stop=(n == ND - 1),
                            )
                        th = thp.tile([128, TW], BF16)
                        nc.scalar.activation(
                            th[:],
                            ps[:],
                            AF.Tanh,
                            bias=hq_sb[:, b * NU + u : b * NU + u + 1],
                        )
                        ths.append(th)
                    for q in range(TW // 128):
                        j = w * (TW // 128) + q
                        for u in range(NU):
                            # within-window t for column j=(w,q) is p*4+q:
                            # stride-4 view of the tanh tile
                            lhsT = ths[u].rearrange("u (p q) -> u q p", q=4)[
                                :, q, :
                            ]
                            nc.tensor.matmul(
                                score_ps[:, j : j + 1],
                                lhsT,
                                vv_sb[:, u : u + 1],
                                start=(u == 0),
                                stop=(u == NU - 1),
                                skip_group_check=True,
                            )

                # ---- softmax over all T (partition x col layout) ----
                exp_sb = smallp.tile([128, NCH], F32)
                rowsum = smallp.tile([128, 1], F32)
                nc.scalar.activation(
                    exp_sb[:], score_ps[:], AF.Exp, accum_out=rowsum[:]
                )
                tot_ps = miscp.tile([1, 1], F32, tag="small_ps", padded_shape=[128, 1])
                nc.tensor.matmul(
                    tot_ps[:], ones_col[:], rowsum[:], start=True, stop=True
                )
                inv_sb = smallp.tile([1, 1], F32)
                nc.vector.reciprocal(inv_sb[:], tot_ps[:])
                bcast_ps = miscp.tile([128, 1], F32, tag="small_ps")
                nc.tensor.matmul(
                    bcast_ps[:], ones_row[:], inv_sb[:], start=True, stop=True
                )
                inv_bc = smallp.tile([128, 1], F32)
                nc.vector.tensor_copy(inv_bc[:], bcast_ps[:])
                attn_f = smallp.tile([128, NCH], F32)
                nc.vector.tensor_scalar_mul(attn_f[:], exp_sb[:], inv_bc[:])
                attn_b = smallp.tile([128, NCH], BF16)
                nc.vector.tensor_scalar_mul(attn_b[:], exp_sb[:], inv_bc[:])
                nc.scalar.dma_start(attn_out[b], attn_f[:])

                # ---- context ----
                # 4 concurrent M=1 matmuls in separate 32-col PE groups
                # (tile_position col-tiling); partial sums land on psum
                # partitions 0/32/64/96 and are summed on DVE.
                ctx_ps = miscp.tile([128, D], F32, tag="ctx_ps")
                NG = 4
                for j in range(NCH):
                    g = j % NG
                    nc.tensor.matmul(
                        ctx_ps[32 * g : 32 * g + 1, :],
                        attn_b[:, j : j + 1],
                        vn_sb[:, j * D : (j + 1) * D],
                        start=(j < NG),
                        stop=(j >= NCH - NG),
                        tile_position=(0, 32 * g),
                        skip_group_check=True,
                    )
                # only one PSUM operand allowed per DVE op: copy then chain adds
                ctx_sb = smallp.tile([1, D], F32)
                nc.vector.tensor_copy(ctx_sb[:], ctx_ps[0:1, :])
                for g in range(1, NG):
                    nc.vector.tensor_tensor(
                        ctx_sb[:], ctx_sb[:], ctx_ps[32 * g : 32 * g + 1, :],
                        mybir.AluOpType.add,
                    )
                nc.scalar.dma_start(ctx_out[b], ctx_sb[:])

    nc.compile()
    return nc


def get_nc():
    if "nc" not in _cache:
        _cache["nc"] = build_nc()
    return _cache["nc"]


def make_in_maps(values, query, W1, b1, W2, b2, V, bV):
    values = np.asarray(values, dtype=np.float32)
    query = np.asarray(query, dtype=np.float32)
    W1 = np.asarray(W1, dtype=np.float32)
    b1 = np.asarray(b1, dtype=np.float32)
    W2 = np.asarray(W2, dtype=np.float32)
    b2 = np.asarray(b2, dtype=np.float32)
    V = np.asarray(V, dtype=np.float32)

    hq_all = query @ W2 + b2 + b1  # [B, U] fp32
    w1_bf = W1.astype(ml_dtypes.bfloat16)
    vv_dev = np.ascontiguousarray(V.reshape(NU, 128).T.astype(ml_dtypes.bfloat16))

    in_maps = []
    for c in range(NCORES):
        sl = slice(c * NB, (c + 1) * NB)
        vals_c = values[sl]
        vT_c = np.ascontiguousarray(vals_c.transpose(0, 2, 1)).astype(
            ml_dtypes.bfloat16
        )
        vN_c = vals_c.astype(ml_dtypes.bfloat16)
        hq_c = hq_all[sl]  # [NB, U]
        # hq_dev[p, b*NU+u] = hq_c[b, u*128+p]
        hq_dev = np.ascontiguousarray(
            hq_c.reshape(NB * NU, 128).T.astype(np.float32)
        )
        in_maps.append(
            {
                "vT": vT_c,
                "vN": vN_c,
                "w1": w1_bf,
                "hq": hq_dev,
                "vv": vv_dev,
            }
        )
    return in_maps


def postprocess(results):
    ctx = np.concatenate([r["ctx_out"] for r in results], axis=0).astype(np.float32)
    # attn_out[b, p, w*4+q] holds weight for t = w*512 + p*4 + q
    attn = np.concatenate(
        [
            r["attn_out"]
            .reshape(NB, 128, NTW, 4)
            .transpose(0, 2, 1, 3)
            .reshape(NB, T, 1)
            for r in results
        ],
        axis=0,
    ).astype(np.float32)
    return ctx, attn


def kernel(values, query, W1, b1, W2, b2, V, bV):
    nc = get_nc()
    in_maps = make_in_maps(values, query, W1, b1, W2, b2, V, bV)
    res = run_bass_kernel_spmd(nc, in_maps, list(range(NCORES)))
    return postprocess(res.results)
